# revision 1
# baseline (speedup 1.0000x reference)
"""Positional-encoding add kernel for Trainium2 (8 NeuronCores, SPMD).

Problem: X[4, 4096, 2048] f32; out = X + PE[None, :, :] where
  PE[s, 2i]   = sin(s / 10000^(2i/2048))
  PE[s, 2i+1] = cos(s / 10000^(2i/2048))

Sharding: sequence dim split 8 ways -> 512 positions per core.
Per core the shard is [4, 512, 2048] = 16 MiB, flattened to rows
[2048, 2048] (row = b*512 + s_local).  The 4 MiB PE shard for the
core's 512 positions is loaded once into SBUF and reused for all 4
batches; X streams through in 4 MiB [128, 8192] tiles (one batch
each, partition p holding positions 4p..4p+3).

This is purely memory-bound: 32 MiB X traffic + 4 MiB PE per core.
"""

import os

import numpy as np

B, S, D = 4, 4096, 2048
N_CORES = 8
S_SHARD = S // N_CORES          # 512 positions per core
ROWS = B * S_SHARD              # 2048 rows per core
P = 128                         # SBUF partitions
R = S_SHARD // P                # 4 positions per partition
FREE = R * D                    # 8192 free elems per partition

_cached_nc = None
LAST_RESULT = None              # BassKernelResults of the last run (for test.py)


def _build_nc(repeat: int = 1):
    import concourse.bacc as bacc
    import concourse.mybir as mybir
    from concourse.tile import TileContext

    f32 = mybir.dt.float32
    nc = bacc.Bacc(None, target_bir_lowering=False, debug=False)
    x = nc.dram_tensor("X", [ROWS, D], f32, kind="ExternalInput")
    pe = nc.dram_tensor("PE", [S_SHARD, D], f32, kind="ExternalInput")
    out = nc.dram_tensor("OUT", [ROWS, D], f32, kind="ExternalOutput")

    # 1 MiB tiles: tile t covers rows [t*128, (t+1)*128); row = 512*b + s_local,
    # so tile t is batch t//4, position block (t%4)*128, matching PE tile t%4.
    n_tiles = ROWS // P          # 16
    n_pe = S_SHARD // P          # 4
    xv = x.rearrange("(t p) d -> t p d", t=n_tiles, p=P)
    ov = out.rearrange("(t p) d -> t p d", t=n_tiles, p=P)
    pev = pe.rearrange("(t p) d -> t p d", t=n_pe, p=P)

    with TileContext(nc) as tc:
        with (
            tc.tile_pool(name="pe", bufs=n_pe) as pe_pool,
            tc.tile_pool(name="xs", bufs=16) as xs_pool,
        ):
            pe_ts = []
            for t in range(n_pe):
                pt = pe_pool.tile([P, D], f32)
                # SWDGE ring for PE so the sync ring starts X loads at t=0
                nc.gpsimd.dma_start(out=pt, in_=pev[t])
                pe_ts.append(pt)
            for _rep in range(repeat):
                for t in range(n_tiles):
                    xt = xs_pool.tile([P, D], f32)
                    nc.sync.dma_start(out=xt, in_=xv[t])
                    # fp32 tensor_tensor runs at 1x on DVE (no 2x uop); offload
                    # every 3rd add to GpSimd (~2x slower) to balance engines
                    eng = nc.gpsimd if t % 3 == 2 else nc.vector
                    eng.tensor_add(out=xt, in0=xt, in1=pe_ts[t % n_pe])
                    nc.sync.dma_start(out=ov[t], in_=xt)
    nc.finalize()
    return nc


def _pe_table() -> np.ndarray:
    """PE table [S, D] f32, matching the jax-on-CPU f32 reference bitwise."""
    try:
        import jax

        with jax.default_device(jax.devices("cpu")[0]):
            import jax.numpy as jnp

            pos = jnp.arange(S, dtype=jnp.float32)[:, None]
            i = jnp.arange(D // 2, dtype=jnp.float32)[None, :]
            angle = pos / jnp.power(jnp.asarray(10000.0, jnp.float32), 2.0 * i / D)
            pe = jnp.stack([jnp.sin(angle), jnp.cos(angle)], axis=-1)
            return np.asarray(pe.reshape(S, D), dtype=np.float32)
    except Exception:
        pos = np.arange(S, dtype=np.float32)[:, None]
        i = np.arange(D // 2, dtype=np.float32)[None, :]
        expo = ((np.float32(2.0) * i) / np.float32(D)).astype(np.float32)
        denom = np.power(np.float32(10000.0), expo, dtype=np.float32)
        angle = (pos / denom).astype(np.float32)
        pe = np.stack(
            [np.sin(angle, dtype=np.float32), np.cos(angle, dtype=np.float32)],
            axis=-1,
        )
        return np.ascontiguousarray(pe.reshape(S, D), dtype=np.float32)


def kernel(X: np.ndarray) -> np.ndarray:
    global _cached_nc, LAST_RESULT
    from concourse.bass_utils import run_bass_kernel_spmd

    X = np.asarray(X)
    assert X.shape == (B, S, D), X.shape
    X = np.ascontiguousarray(X, dtype=np.float32)

    if _cached_nc is None:
        _cached_nc = _build_nc()
    nc = _cached_nc

    pe = _pe_table()
    in_maps = []
    for c in range(N_CORES):
        xs = np.ascontiguousarray(X[:, c * S_SHARD : (c + 1) * S_SHARD, :]).reshape(
            ROWS, D
        )
        pes = np.ascontiguousarray(pe[c * S_SHARD : (c + 1) * S_SHARD, :])
        in_maps.append({"X": xs, "PE": pes})

    trace = bool(int(os.environ.get("KERNEL_TRACE", "0")))
    res = run_bass_kernel_spmd(
        nc, in_maps, core_ids=list(range(N_CORES)), trace=trace
    )
    LAST_RESULT = res

    out = np.empty((B, S, D), dtype=np.float32)
    for c in range(N_CORES):
        out[:, c * S_SHARD : (c + 1) * S_SHARD, :] = res.results[c]["OUT"].reshape(
            B, S_SHARD, D
        )
    return out

